# revision 25
# baseline (speedup 1.0000x reference)
"""BitNet dense layer on 8 Trainium2 NeuronCores.

reference math:
    row_scale = clip(mean(|W|, axis=1), 1e-8)        # [out]
    out = (x @ sign(W).T) * row_scale * scale_param  # [B,S,out]

Strategy (data-parallel over the 8192 tokens, split-K mixed precision):
  * The binary weight is exactly representable in fp8 (+-1), and the 2e-2
    error budget is ~17x the bf16 activation error, so most of the
    contraction dim runs through the fp8 DoubleRow path (157 TF/s, 2x bf16):
        out = x8[:, :K8] @ S8 + xb[:, K8:] @ Sb      (sign domain, fp32 psum)
    with x8 = e4m3(x), xb = bf16(x). Host applies the exact fp32 row scale
    afterwards:  out *= row_scale * scale_param
  * Error correction through the bf16 columns: the fp8 quantization residual
    d = x8 - x produces output error e = d @ S8^T. Since the bf16 half is
    transmitted near-exactly, perturbing it by the least-squares solution of
    Sb delta = -S8 d (delta = -d @ M^T, M = (Sb^T Sb)^-1 Sb^T S8, computed
    once on host) cancels the projection of e onto span(Sb) - a KB/4096
    fraction of the error power. Error then scales ~linearly in K8 instead
    of sqrt(K8): K8=3840/KB=256 measures max_rel 0.0180 (vs 0.0212 for
    uncorrected full-fp8 K8=4096, over the 2e-2 budget), with the fp8
    stream covering 15/16 of the contraction.
  * All K-batches run as ONE composable tile matmul (fp8 split 3584+256 so
    the bulk keeps 512-row k-tiles with even DoubleRow pairing), so fp8 and
    bf16 partials accumulate into the same PSUM group: single output
    tensor, no inter-kernel bubble, one eviction tail.
  * Output is written as bf16 and upcast on host: simulated max_rel is
    0.01794 (the 2^-9 rounding is negligible against the fp8 budget), and
    it halves both the PSUM-eviction write cost and the output DMA bytes,
    shortening the final evict->DMA tail.
  * PSUM double-buffered (2 x 4 banks) so evictions overlap the next
    n-tile's accumulation.
  * Host pre-TILES operands into the exact SBUF tile layout
    ([kt, (mt|nt), 128, k_subtiles, 512], partition-major) so every tile
    DMA is one contiguous 128-256KB block (2KB DMA packets instead of the
    512B packets a strided [K, M] view produces).
  * kxm tiles issue from GpSimd (idle otherwise) so first-wave x/w tile
    issues run in two parallel lanes; spreading further across engines
    measured SLOWER (cross-engine ordering sems stall the ramp). The Tile
    teardown (~12.5us: all-engine barriers + per-engine clears of the full
    semaphore pools) measured CONSTANT across pool/engine layouts - don't
    bother restructuring to shrink it.
  * Warmup matmuls release the HAM clock gate (1.2 -> 2.4 GHz needs ~3.4us
    of PE activity) while the first tiles DMA in, sized to end right as
    they land (~12us).
"""

import numpy as np
import ml_dtypes

B, S, D_IN, D_OUT = 4, 2048, 4096, 4096
N_CORES = 8
M_TOT = B * S
M_LOC = M_TOT // N_CORES
K8 = 3840  # contraction columns routed through fp8 DoubleRow
K8A = 3584  # ... split as 3584 (512-row k-tiles) + 256 so the bulk of the
K8B = 256   # fp8 stream keeps the most efficient tile width
KB = D_IN - K8
N_WARMUP = 8

_prog = None
last_results = None  # BassKernelResults of the most recent run (for test harness)
TRACE = False  # set True by the dev test harness (needs NTFF shims) to profile


def _build_program():
    import concourse.tile as tile
    from concourse import bacc, mybir
    from concourse.kernels.tile_matmul import (
        ShapeInfo,
        composable_matmul_tile_kernel,
    )

    nc = bacc.Bacc(
        "TRN2", target_bir_lowering=False, debug=False, num_devices=N_CORES
    )
    # Pre-tiled operands: each [.., 128, k_subtiles, 512] block is one
    # contiguous DRAM region matching the SBUF tile layout exactly.
    xa = nc.dram_tensor(
        "xa", [K8A // 512, M_LOC // 512, 128, 4, 512], mybir.dt.float8e4,
        kind="ExternalInput",
    ).ap()
    xb8 = nc.dram_tensor(
        "xb8", [M_LOC // 512, 128, 2, 512], mybir.dt.float8e4,
        kind="ExternalInput",
    ).ap()
    xbb = nc.dram_tensor(
        "xbb", [M_LOC // 512, 128, 2, 512], mybir.dt.bfloat16,
        kind="ExternalInput",
    ).ap()
    wa = nc.dram_tensor(
        "wa", [K8A // 512, D_OUT // 512, 128, 4, 512], mybir.dt.float8e4,
        kind="ExternalInput",
    ).ap()
    wb8 = nc.dram_tensor(
        "wb8", [D_OUT // 512, 128, 2, 512], mybir.dt.float8e4,
        kind="ExternalInput",
    ).ap()
    wbb = nc.dram_tensor(
        "wbb", [D_OUT // 512, 128, 2, 512], mybir.dt.bfloat16,
        kind="ExternalInput",
    ).ap()
    out = nc.dram_tensor(
        "out", [M_LOC, D_OUT], mybir.dt.bfloat16, kind="ExternalOutput"
    ).ap()
    with tile.TileContext(nc) as tc:
        # PE warmup: dummy matmuls run while the first real tiles DMA in,
        # releasing the HAM clock gate so the real matmul stream starts at
        # full clock right as the first tiles land. PE executes in order,
        # so a longer warmup would gate the real stream on itself.
        with (
            tc.tile_pool(name="warm", bufs=1) as warm,
            tc.tile_pool(name="warm_psum", bufs=1, space="PSUM") as warm_psum,
        ):
            wrm_a = warm.tile([128, 128], mybir.dt.bfloat16)
            wrm_b = warm.tile([128, 512], mybir.dt.bfloat16)
            # Memsets on GPSIMD: it comes out of the engine preamble ~1.5us
            # before DVE's first slot, so the warmup matmuls (which wait on
            # these) issue that much earlier. (Skipping initialization
            # entirely trips the CoreSim read-before-write check.)
            nc.gpsimd.memset(wrm_a[:], 0.0)
            nc.gpsimd.memset(wrm_b[:], 0.0)
            ps = warm_psum.tile([128, 512], mybir.dt.float32)
            for i in range(N_WARMUP):
                nc.tensor.matmul(
                    ps[:], wrm_a[:], wrm_b[:],
                    start=(i == 0), stop=(i == N_WARMUP - 1),
                )

        tc.swap_default_side()
        with (
            tc.tile_pool(name="kxm", bufs=4) as kxm_pool,
            tc.tile_pool(name="kxn", bufs=4) as kxn_pool,
        ):
            kxm_shape = ShapeInfo(
                pdims=((128, K8A // 128), (128, 2), (128, 2)), fdims=(M_LOC,)
            )
            kxn_shape = ShapeInfo(
                pdims=((128, K8A // 128), (128, 2), (128, 2)), fdims=(D_OUT,)
            )

            def kxm_producer(nc_, md):
                if md.k_batch_idx == 0:
                    t = kxm_pool.tile(
                        [128, 4, 512], mybir.dt.float8e4, tag="kxma", bufs=12
                    )
                    nc_.gpsimd.dma_start(t[:], xa[md.k_tile_idx, md.m_tile_idx])
                elif md.k_batch_idx == 1:
                    t = kxm_pool.tile(
                        [128, 2, 512], mybir.dt.float8e4, tag="kxmb8", bufs=3
                    )
                    nc_.gpsimd.dma_start(t[:], xb8[md.m_tile_idx])
                else:
                    t = kxm_pool.tile(
                        [128, 2, 512], mybir.dt.bfloat16, tag="kxmbb", bufs=3
                    )
                    nc_.gpsimd.dma_start(t[:], xbb[md.m_tile_idx])
                return t

            def kxn_producer(nc_, md):
                if md.k_batch_idx == 0:
                    t = kxn_pool.tile(
                        [128, 4, 512], mybir.dt.float8e4, tag="kxna", bufs=12
                    )
                    nc_.sync.dma_start(t[:], wa[md.k_tile_idx, md.n_tile_idx])
                elif md.k_batch_idx == 1:
                    t = kxn_pool.tile(
                        [128, 2, 512], mybir.dt.float8e4, tag="kxnb8", bufs=3
                    )
                    nc_.sync.dma_start(t[:], wb8[md.n_tile_idx])
                else:
                    t = kxn_pool.tile(
                        [128, 2, 512], mybir.dt.bfloat16, tag="kxnbb", bufs=3
                    )
                    nc_.sync.dma_start(t[:], wbb[md.n_tile_idx])
                return t

            from concourse.bass import ds

            out3d = out.rearrange("(po pi) f -> pi po f", pi=128)

            def consumer(nc_, mxn_tile, md):
                # One DMA per m-subtile instead of one per tile: each write
                # depends only on its own subtile's eviction, so the final
                # evict->DMA chain pipelines instead of serializing.
                for i in range(mxn_tile.shape[1]):
                    nc_.sync.dma_start(
                        out3d[
                            :,
                            md.m_tile_idx * md.m_subtiles + i,
                            ds(md.n_tile_idx * md.n_tile, md.n_tile),
                        ],
                        mxn_tile[:, i, : md.n_tile],
                    )

            def reducer(nc_, psum, sbuf, md):
                # PSUM evictions (fp32 psum -> bf16 sbuf, cast in the copy)
                # alternate between DVE and ACT (GPSIMD cannot read PSUM) so
                # consecutive evictions run in parallel.
                if md.m_subtile_idx % 2 == 0:
                    nc_.vector.tensor_copy(out=sbuf, in_=psum)
                else:
                    nc_.scalar.activation(
                        sbuf, psum, mybir.ActivationFunctionType.Copy
                    )

            composable_matmul_tile_kernel(
                tc=tc,
                kxm_shape=kxm_shape,
                kxn_shape=kxn_shape,
                output_type=mybir.dt.bfloat16,
                kxm_producer=kxm_producer,
                kxn_producer=kxn_producer,
                mxn_consumer=consumer,
                mxn_subtile_reducer=reducer,
                psum_n_bufs=2,
            )
    nc.compile()
    return nc


def _tile_kxm(a2d, sub):
    """[K, M] (K = kt*sub*128) -> [kt, M//512, 128, sub, 512] contiguous."""
    K, M = a2d.shape
    kt = K // (sub * 128)
    return np.ascontiguousarray(
        a2d.reshape(kt, sub, 128, M // 512, 512).transpose(0, 3, 2, 1, 4)
    )


def kernel(input, weight, scale_param):
    global _prog, last_results
    from concourse.bass_utils import run_bass_kernel_spmd

    x = np.asarray(input, dtype=np.float32).reshape(M_TOT, D_IN)
    W = np.asarray(weight, dtype=np.float32)
    sp = np.asarray(scale_param, dtype=np.float32)

    comb = np.clip(np.abs(W).mean(axis=1, dtype=np.float32), 1e-8, None) * sp
    ST = np.sign(W).T  # [in, out], exact +-1/0
    wT8 = ST[:K8].astype(ml_dtypes.float8_e4m3, order="C")
    wTb = ST[K8:].astype(ml_dtypes.bfloat16, order="C")

    x8 = x[:, :K8].astype(ml_dtypes.float8_e4m3)
    # Least-squares cancellation of the fp8 residual through the bf16
    # columns (see module docstring): delta = (x - x8) @ M^T.
    S8 = ST[:K8]  # [K8, out] = S8^T
    SB = ST[K8:]  # [KB, out] = Sb^T
    G = (SB @ SB.T).astype(np.float64)  # [KB, KB], exact small ints
    C = (SB @ S8.T).astype(np.float64)  # [KB, K8]
    M = np.linalg.solve(G, C).astype(np.float32)  # [KB, K8]
    d = x[:, :K8] - x8.astype(np.float32)
    xbf = x[:, K8:] + d @ M.T

    xT8 = np.ascontiguousarray(x8.T)
    xTb = xbf.T.astype(ml_dtypes.bfloat16, order="C")

    # Shared (weight) tiles, one copy for all cores.
    wa_t = _tile_kxm(wT8[:K8A], 4)          # [7, 8, 128, 4, 512]
    wb8_t = _tile_kxm(wT8[K8A:], 2)[0]      # [8, 128, 2, 512]
    wbb_t = _tile_kxm(wTb, 2)[0]            # [8, 128, 2, 512]

    if _prog is None:
        _prog = _build_program()

    in_maps = []
    for c in range(N_CORES):
        sl = slice(c * M_LOC, (c + 1) * M_LOC)
        in_maps.append(
            {
                "xa": _tile_kxm(xT8[:K8A, sl], 4),      # [7, 2, 128, 4, 512]
                "xb8": _tile_kxm(xT8[K8A:, sl], 2)[0],  # [2, 128, 2, 512]
                "xbb": _tile_kxm(xTb[:, sl], 2)[0],     # [2, 128, 2, 512]
                "wa": wa_t,
                "wb8": wb8_t,
                "wbb": wbb_t,
            }
        )
    last_results = run_bass_kernel_spmd(
        _prog, in_maps, list(range(N_CORES)), trace=TRACE
    )
    out = np.concatenate(
        [last_results.results[c]["out"] for c in range(N_CORES)], axis=0
    ).astype(np.float32)
    out *= comb[None, :]
    return np.nan_to_num(
        out.reshape(B, S, D_OUT), nan=0.0, posinf=1e6, neginf=-1e6
    )


# revision 26
# speedup vs baseline: 1.0084x; 1.0084x over previous
"""BitNet dense layer on 8 Trainium2 NeuronCores.

reference math:
    row_scale = clip(mean(|W|, axis=1), 1e-8)        # [out]
    out = (x @ sign(W).T) * row_scale * scale_param  # [B,S,out]

Strategy (data-parallel over the 8192 tokens, split-K mixed precision):
  * The binary weight is exactly representable in fp8 (+-1), and the 2e-2
    error budget is ~17x the bf16 activation error, so most of the
    contraction dim runs through the fp8 DoubleRow path (157 TF/s, 2x bf16):
        out = x8[:, :K8] @ S8 + xb[:, K8:] @ Sb      (sign domain, fp32 psum)
    with x8 = e4m3(x), xb = bf16(x). Host applies the exact fp32 row scale
    afterwards:  out *= row_scale * scale_param
  * Error correction through the bf16 columns: the fp8 quantization residual
    d = x8 - x produces output error e = d @ S8^T. Since the bf16 half is
    transmitted near-exactly, perturbing it by the least-squares solution of
    Sb delta = -S8 d (delta = -d @ M^T, M = (Sb^T Sb)^-1 Sb^T S8, computed
    once on host) cancels the projection of e onto span(Sb) - a KB/4096
    fraction of the error power. Error then scales ~linearly in K8 instead
    of sqrt(K8): K8=3840/KB=256 measures max_rel 0.0180 (vs 0.0212 for
    uncorrected full-fp8 K8=4096, over the 2e-2 budget), with the fp8
    stream covering 15/16 of the contraction.
  * All K-batches run as ONE composable tile matmul (fp8 split 3584+256 so
    the bulk keeps 512-row k-tiles with even DoubleRow pairing), so fp8 and
    bf16 partials accumulate into the same PSUM group: single output
    tensor, no inter-kernel bubble, one eviction tail.
  * Output is written as bf16 and upcast on host: simulated max_rel is
    0.01794 (the 2^-9 rounding is negligible against the fp8 budget), and
    it halves both the PSUM-eviction write cost and the output DMA bytes,
    shortening the final evict->DMA tail.
  * PSUM double-buffered (2 x 4 banks) so evictions overlap the next
    n-tile's accumulation.
  * Host pre-TILES operands into the exact SBUF tile layout
    ([kt, (mt|nt), 128, k_subtiles, 512], partition-major) so every tile
    DMA is one contiguous 128-256KB block (2KB DMA packets instead of the
    512B packets a strided [K, M] view produces).
  * kxm tiles issue from GpSimd (idle otherwise) so first-wave x/w tile
    issues run in two parallel lanes; spreading further across engines
    measured SLOWER (cross-engine ordering sems stall the ramp). The Tile
    teardown (~12.5us: all-engine barriers + per-engine clears of the full
    semaphore pools) measured CONSTANT across pool/engine layouts - don't
    bother restructuring to shrink it.
  * Warmup matmuls release the HAM clock gate (1.2 -> 2.4 GHz needs ~3.4us
    of PE activity) while the first tiles DMA in, sized to end right as
    they land (~12us).
"""

import numpy as np
import ml_dtypes

B, S, D_IN, D_OUT = 4, 2048, 4096, 4096
N_CORES = 8
M_TOT = B * S
M_LOC = M_TOT // N_CORES
K8 = 3840  # contraction columns routed through fp8 DoubleRow
K8A = 3584  # ... split as 3584 (512-row k-tiles) + 256 so the bulk of the
K8B = 256   # fp8 stream keeps the most efficient tile width
KB = D_IN - K8
N_WARMUP = 8

_prog = None
last_results = None  # BassKernelResults of the most recent run (for test harness)
TRACE = False  # set True by the dev test harness (needs NTFF shims) to profile


def _build_program():
    import concourse.tile as tile
    from concourse import bacc, mybir
    from concourse.kernels.tile_matmul import (
        ShapeInfo,
        composable_matmul_tile_kernel,
    )

    nc = bacc.Bacc(
        "TRN2", target_bir_lowering=False, debug=False, num_devices=N_CORES
    )
    # Pre-tiled operands: each [.., 128, k_subtiles, 512] block is one
    # contiguous DRAM region matching the SBUF tile layout exactly.
    xa = nc.dram_tensor(
        "xa", [K8A // 512, M_LOC // 512, 128, 4, 512], mybir.dt.float8e4,
        kind="ExternalInput",
    ).ap()
    xb8 = nc.dram_tensor(
        "xb8", [M_LOC // 512, 128, 2, 512], mybir.dt.float8e4,
        kind="ExternalInput",
    ).ap()
    xbb = nc.dram_tensor(
        "xbb", [M_LOC // 512, 128, 2, 512], mybir.dt.bfloat16,
        kind="ExternalInput",
    ).ap()
    wa = nc.dram_tensor(
        "wa", [K8A // 512, D_OUT // 512, 128, 4, 512], mybir.dt.float8e4,
        kind="ExternalInput",
    ).ap()
    wb8 = nc.dram_tensor(
        "wb8", [D_OUT // 512, 128, 2, 512], mybir.dt.float8e4,
        kind="ExternalInput",
    ).ap()
    wbb = nc.dram_tensor(
        "wbb", [D_OUT // 512, 128, 2, 512], mybir.dt.bfloat16,
        kind="ExternalInput",
    ).ap()
    out = nc.dram_tensor(
        "out", [M_LOC, D_OUT], mybir.dt.bfloat16, kind="ExternalOutput"
    ).ap()
    with tile.TileContext(nc) as tc:
        # PE warmup: dummy matmuls run while the first real tiles DMA in,
        # releasing the HAM clock gate so the real matmul stream starts at
        # full clock right as the first tiles land. PE executes in order,
        # so a longer warmup would gate the real stream on itself.
        with (
            tc.tile_pool(name="warm", bufs=1) as warm,
            tc.tile_pool(name="warm_psum", bufs=1, space="PSUM") as warm_psum,
        ):
            wrm_a = warm.tile([128, 128], mybir.dt.bfloat16)
            wrm_b = warm.tile([128, 512], mybir.dt.bfloat16)
            # Memsets on GPSIMD: it comes out of the engine preamble ~1.5us
            # before DVE's first slot, so the warmup matmuls (which wait on
            # these) issue that much earlier. (Skipping initialization
            # entirely trips the CoreSim read-before-write check.)
            nc.gpsimd.memset(wrm_a[:], 0.0)
            nc.gpsimd.memset(wrm_b[:], 0.0)
            ps = warm_psum.tile([128, 512], mybir.dt.float32)
            for i in range(N_WARMUP):
                nc.tensor.matmul(
                    ps[:], wrm_a[:], wrm_b[:],
                    start=(i == 0), stop=(i == N_WARMUP - 1),
                )

        tc.swap_default_side()
        with (
            tc.tile_pool(name="kxm", bufs=4) as kxm_pool,
            tc.tile_pool(name="kxn", bufs=4) as kxn_pool,
        ):
            kxm_shape = ShapeInfo(
                pdims=((128, K8A // 128), (128, 2), (128, 2)), fdims=(M_LOC,)
            )
            kxn_shape = ShapeInfo(
                pdims=((128, K8A // 128), (128, 2), (128, 2)), fdims=(D_OUT,)
            )

            def kxm_producer(nc_, md):
                if md.k_batch_idx == 0:
                    t = kxm_pool.tile(
                        [128, 4, 512], mybir.dt.float8e4, tag="kxma", bufs=9
                    )
                    nc_.gpsimd.dma_start(t[:], xa[md.k_tile_idx, md.m_tile_idx])
                elif md.k_batch_idx == 1:
                    t = kxm_pool.tile(
                        [128, 2, 512], mybir.dt.float8e4, tag="kxmb8", bufs=3
                    )
                    nc_.gpsimd.dma_start(t[:], xb8[md.m_tile_idx])
                else:
                    t = kxm_pool.tile(
                        [128, 2, 512], mybir.dt.bfloat16, tag="kxmbb", bufs=3
                    )
                    nc_.gpsimd.dma_start(t[:], xbb[md.m_tile_idx])
                return t

            def kxn_producer(nc_, md):
                if md.k_batch_idx == 0:
                    t = kxn_pool.tile(
                        [128, 4, 512], mybir.dt.float8e4, tag="kxna", bufs=9
                    )
                    nc_.sync.dma_start(t[:], wa[md.k_tile_idx, md.n_tile_idx])
                elif md.k_batch_idx == 1:
                    t = kxn_pool.tile(
                        [128, 2, 512], mybir.dt.float8e4, tag="kxnb8", bufs=3
                    )
                    nc_.sync.dma_start(t[:], wb8[md.n_tile_idx])
                else:
                    t = kxn_pool.tile(
                        [128, 2, 512], mybir.dt.bfloat16, tag="kxnbb", bufs=3
                    )
                    nc_.sync.dma_start(t[:], wbb[md.n_tile_idx])
                return t

            from concourse.bass import ds

            out3d = out.rearrange("(po pi) f -> pi po f", pi=128)

            def consumer(nc_, mxn_tile, md):
                # One DMA per m-subtile instead of one per tile: each write
                # depends only on its own subtile's eviction, so the final
                # evict->DMA chain pipelines instead of serializing.
                for i in range(mxn_tile.shape[1]):
                    nc_.sync.dma_start(
                        out3d[
                            :,
                            md.m_tile_idx * md.m_subtiles + i,
                            ds(md.n_tile_idx * md.n_tile, md.n_tile),
                        ],
                        mxn_tile[:, i, : md.n_tile],
                    )

            def reducer(nc_, psum, sbuf, md):
                # PSUM evictions (fp32 psum -> bf16 sbuf, cast in the copy)
                # alternate between DVE and ACT (GPSIMD cannot read PSUM) so
                # consecutive evictions run in parallel.
                if md.m_subtile_idx % 2 == 0:
                    nc_.vector.tensor_copy(out=sbuf, in_=psum)
                else:
                    nc_.scalar.activation(
                        sbuf, psum, mybir.ActivationFunctionType.Copy
                    )

            composable_matmul_tile_kernel(
                tc=tc,
                kxm_shape=kxm_shape,
                kxn_shape=kxn_shape,
                output_type=mybir.dt.bfloat16,
                kxm_producer=kxm_producer,
                kxn_producer=kxn_producer,
                mxn_consumer=consumer,
                mxn_subtile_reducer=reducer,
                psum_n_bufs=2,
            )
    nc.compile()
    return nc


def _tile_kxm(a2d, sub):
    """[K, M] (K = kt*sub*128) -> [kt, M//512, 128, sub, 512] contiguous."""
    K, M = a2d.shape
    kt = K // (sub * 128)
    return np.ascontiguousarray(
        a2d.reshape(kt, sub, 128, M // 512, 512).transpose(0, 3, 2, 1, 4)
    )


def kernel(input, weight, scale_param):
    global _prog, last_results
    from concourse.bass_utils import run_bass_kernel_spmd

    x = np.asarray(input, dtype=np.float32).reshape(M_TOT, D_IN)
    W = np.asarray(weight, dtype=np.float32)
    sp = np.asarray(scale_param, dtype=np.float32)

    comb = np.clip(np.abs(W).mean(axis=1, dtype=np.float32), 1e-8, None) * sp
    ST = np.sign(W).T  # [in, out], exact +-1/0
    wT8 = ST[:K8].astype(ml_dtypes.float8_e4m3, order="C")
    wTb = ST[K8:].astype(ml_dtypes.bfloat16, order="C")

    x8 = x[:, :K8].astype(ml_dtypes.float8_e4m3)
    # Least-squares cancellation of the fp8 residual through the bf16
    # columns (see module docstring): delta = (x - x8) @ M^T.
    S8 = ST[:K8]  # [K8, out] = S8^T
    SB = ST[K8:]  # [KB, out] = Sb^T
    G = (SB @ SB.T).astype(np.float64)  # [KB, KB], exact small ints
    C = (SB @ S8.T).astype(np.float64)  # [KB, K8]
    M = np.linalg.solve(G, C).astype(np.float32)  # [KB, K8]
    d = x[:, :K8] - x8.astype(np.float32)
    xbf = x[:, K8:] + d @ M.T

    xT8 = np.ascontiguousarray(x8.T)
    xTb = xbf.T.astype(ml_dtypes.bfloat16, order="C")

    # Shared (weight) tiles, one copy for all cores.
    wa_t = _tile_kxm(wT8[:K8A], 4)          # [7, 8, 128, 4, 512]
    wb8_t = _tile_kxm(wT8[K8A:], 2)[0]      # [8, 128, 2, 512]
    wbb_t = _tile_kxm(wTb, 2)[0]            # [8, 128, 2, 512]

    if _prog is None:
        _prog = _build_program()

    in_maps = []
    for c in range(N_CORES):
        sl = slice(c * M_LOC, (c + 1) * M_LOC)
        in_maps.append(
            {
                "xa": _tile_kxm(xT8[:K8A, sl], 4),      # [7, 2, 128, 4, 512]
                "xb8": _tile_kxm(xT8[K8A:, sl], 2)[0],  # [2, 128, 2, 512]
                "xbb": _tile_kxm(xTb[:, sl], 2)[0],     # [2, 128, 2, 512]
                "wa": wa_t,
                "wb8": wb8_t,
                "wbb": wbb_t,
            }
        )
    last_results = run_bass_kernel_spmd(
        _prog, in_maps, list(range(N_CORES)), trace=TRACE
    )
    out = np.concatenate(
        [last_results.results[c]["out"] for c in range(N_CORES)], axis=0
    ).astype(np.float32)
    out *= comb[None, :]
    return np.nan_to_num(
        out.reshape(B, S, D_OUT), nan=0.0, posinf=1e6, neginf=-1e6
    )
